# revision 8
# baseline (speedup 1.0000x reference)
# DenseGATv2Conv Trainium2 kernel (v2).
#
# Math (per batch b):
#   xl = x @ W_l + b_l ; xr = x @ W_r + b_r            [N, H*C]
#   alpha[i,j,h] = sum_c att[h,c] * leaky_relu(xl[j,hc] + xr[i,hc], 0.2)
#   S = softmax_j(alpha masked by adj(+self loops))
#   out[i,hc] = sum_j S[i,j,h] * xr[j,hc] + bias
#
# Identities used on device:
#   leaky_relu(z) = 0.2*z + 0.8*relu(z)
#   alpha[i,j,h] = 0.2*sl[j,h] + 0.2*sr[i,h] + 0.8*sum_c att[h,c]*relu(xl[j,hc]+xr[i,hc])
# exp(0.2*sr[i,h]) cancels in the softmax; exp(0.2*sl[j,h]) (= esl) is folded
# multiplicatively into the aggregation operand.  The adjacency mask is applied
# ADDITIVELY pre-exp as 30*(adj-1) accumulated into the score PSUM by a small
# matmul, so masked entries underflow to 0 in the fp16 exp output.
#
# Per core: 256 dest rows = 2 ib x 4 supers x 32 rows.  Per super the 16
# dest-row pairs all accumulate into ONE [128, 1024] PSUM tile using 4
# stationary "variants" (att columns at local offset 8v) x 4 tile positions,
# so PSUM row r = 32q + 8v + 4d + h and dest-in-core = sup*32 + 8q + 2v + d
# comes out in natural order.  One exp per super writes fp16 scores which a
# DMA crossbar transpose scatters straight into the S^T aggregation layout.
#
# Sharding: 8 cores = (batch b in 0..1) x (4 blocks of 256 destination rows).

import numpy as np

B, N, F, H, C = 2, 1024, 128, 4, 16
HC = H * C
NCORES = 8
NI = 256          # destination rows per core
NSUP = 8          # supers of 16 pairs (32 dest rows) each
MASK_NEG = 30.0   # additive mask magnitude: exp(score-30) underflows fp16

_CACHE = {}
LAST_RESULTS = None


def _build_program():
    import concourse.bass as bass
    import concourse.mybir as mybir
    import concourse.tile as tile
    from concourse import bacc

    f32 = mybir.dt.float32
    f16 = mybir.dt.float16
    Alu = mybir.AluOpType
    Act = mybir.ActivationFunctionType

    nc = bacc.Bacc(
        "TRN2",
        target_bir_lowering=False,
        debug=False,
        enable_asserts=False,
        num_devices=NCORES,
    )

    # ---- DRAM I/O ----
    xb16 = nc.dram_tensor("xb16", [N, F], f16, kind="ExternalInput").ap()
    xis16 = nc.dram_tensor("xis16", [NI, F], f16, kind="ExternalInput").ap()
    adjh2 = nc.dram_tensor("adjh2", [32, NSUP * N], f16, kind="ExternalInput").ap()
    wl16 = nc.dram_tensor("wl16", [F, HC], f16, kind="ExternalInput").ap()
    wr16 = nc.dram_tensor("wr16", [F, HC], f16, kind="ExternalInput").ap()
    blp = nc.dram_tensor("blp", [HC, 1], f32, kind="ExternalInput").ap()
    brp = nc.dram_tensor("brp", [HC, 1], f32, kind="ExternalInput").ap()
    attv = nc.dram_tensor("attv", [F, 128], f16, kind="ExternalInput").ap()
    p1m = nc.dram_tensor("p1m", [32, 128], f16, kind="ExternalInput").ap()
    attbp = nc.dram_tensor("attbp", [HC, 16], f16, kind="ExternalInput").ap()
    biasb = nc.dram_tensor("biasb", [128, HC], f32, kind="ExternalInput").ap()
    out = nc.dram_tensor("out", [NI, HC], f32, kind="ExternalOutput").ap()

    with tile.TileContext(nc) as tc:
        _body(tc, nc, mybir, f32, f16, Alu, Act,
              xb16, xis16, adjh2, wl16, wr16, blp, brp, attv, p1m, attbp,
              biasb, out)

    nc.compile()
    return nc


def _body(tc, nc, mybir, f32, f16, Alu, Act,
          xb16, xis16, adjh2, wl16, wr16, blp, brp, attv, p1m, attbp,
          biasb, out):
    from contextlib import ExitStack
    ctx = ExitStack()
    with ctx:
        consts = ctx.enter_context(tc.tile_pool(name="consts", bufs=1))
        work = ctx.enter_context(tc.tile_pool(name="work", bufs=1))
        rp_pool = ctx.enter_context(tc.tile_pool(name="rp", bufs=32))
        sc_pool = ctx.enter_context(tc.tile_pool(name="sc", bufs=2))
        outp = ctx.enter_context(tc.tile_pool(name="outp", bufs=2))
        psg = ctx.enter_context(tc.tile_pool(name="psg", bufs=2, space="PSUM"))
        psa = ctx.enter_context(tc.tile_pool(name="psa", bufs=2, space="PSUM"))

        dma = nc.sync.dma_start
        dma2 = nc.scalar.dma_start          # second HWDGE queue for constants
        dmaT = nc.sync.dma_start_transpose

        # ---------- x^T via DMA crossbar transpose (DRAM -> SBUF) ----------
        xT = consts.tile([F, N], f16, tag="xT")       # [f, node]
        xisT = consts.tile([F, NI], f16, tag="xisT")  # [f, dest-slice node]
        dmaT(xT[:].rearrange("p (k a) -> p k a", a=128), xb16)
        dmaT(xisT[:].rearrange("p (k a) -> p k a", a=128), xis16)

        # ---------- constants ----------
        wl_t = consts.tile([F, HC], f16, tag="wl")
        wr_t = consts.tile([F, HC], f16, tag="wr")
        blp_t = consts.tile([HC, 1], f32, tag="blp")
        brp_t = consts.tile([HC, 1], f32, tag="brp")
        attv_t = consts.tile([F, 128], f16, tag="attv")
        p1_t = consts.tile([32, 128], f16, tag="p1")
        attbp_t = consts.tile([HC, 16], f16, tag="attbp")
        biasb_t = consts.tile([128, HC], f32, tag="biasb")
        adjh_t = consts.tile([32, NSUP * N], f16, tag="adjh")
        dma2(wl_t[:], wl16)
        dma2(wr_t[:], wr16)
        dma2(blp_t[:], blp)
        dma2(brp_t[:], brp)
        dma2(attv_t[:], attv)
        dma(adjh_t[:], adjh2)
        dma2(p1_t[:], p1m)
        dma2(attbp_t[:], attbp)
        dma2(biasb_t[:], biasb)

        # ---------- projections ----------
        # xl2T: (x@W_l+b_l)^T stacked twice on partitions (for pair bias adds)
        xl2T = consts.tile([128, N], f16, tag="xl2T")
        xrT16 = consts.tile([HC, N], f16, tag="xrT16")   # (x@W_r+b_r)^T
        xrsT = consts.tile([HC, NI], f32, tag="xrsT")    # dest-row slice, f32
        pj = psg.tile([HC, N], f32, tag="g", name="pj")
        for half in range(2):
            s = slice(half * 512, (half + 1) * 512)
            nc.tensor.matmul(pj[:, s], wl_t[:], xT[:, s], start=True, stop=True)
        pj3 = psa.tile([HC, NI], f32, tag="a", name="pj3")
        nc.tensor.matmul(pj3[:], wr_t[:], xisT[:], start=True, stop=True)
        nc.scalar.activation(xl2T[0:HC, :], pj[:], Act.Identity,
                             bias=blp_t[:, 0:1], scale=1.0)
        nc.scalar.activation(xl2T[HC:128, :], pj[:], Act.Identity,
                             bias=blp_t[:, 0:1], scale=1.0)
        nc.scalar.activation(xrsT[:], pj3[:], Act.Identity,
                             bias=brp_t[:, 0:1], scale=1.0)
        pj2 = psg.tile([HC, N], f32, tag="g", name="pj2")
        for half in range(2):
            s = slice(half * 512, (half + 1) * 512)
            nc.tensor.matmul(pj2[:, s], wr_t[:], xT[:, s], start=True, stop=True)
        nc.scalar.activation(xrT16[:], pj2[:], Act.Identity,
                             bias=brp_t[:, 0:1], scale=1.0)

        # ---------- xrp: per-pair bias columns [xr[2p] ; xr[2p+1]] ----------
        xrp = consts.tile([128, 128], f32, tag="xrp")
        ev = xrsT[:].rearrange("p (a two) -> p a two", two=2)
        nc.vector.tensor_copy(xrp[0:HC, :], ev[:, :, 0])
        nc.vector.tensor_copy(xrp[HC:128, :], ev[:, :, 1])

        # ---------- xr_mod build: [j128, k, h, 0:16]=xr*esl, [..,16]=esl ----
        def build_xr_mod():
            # sl[h,j] = sum_hc att_blk[hc,h]*xl[hc,j]; esl = exp(0.2*sl)
            psl = psa.tile([16, N], f32, tag="a", name="psl")
            for half in range(2):
                s = slice(half * 512, (half + 1) * 512)
                nc.tensor.matmul(psl[:, s], attbp_t[:], xl2T[0:HC, s],
                                 start=True, stop=True)
            eslT = work.tile([16, N], f16, tag="eslT", name="eslT")
            nc.scalar.activation(eslT[:], psl[:], Act.Exp, scale=0.2)
            xr_nat = work.tile([128, 8 * HC], f16, tag="xrnat", name="xr_nat")
            esln = work.tile([128, 8 * 16], f16, tag="esln", name="esln")
            dmaT(xr_nat[:].rearrange("p (k c) -> p k c", k=8), xrT16[:])
            dmaT(esln[:].rearrange("p (k e) -> p k e", k=8), eslT[:])
            xmv = xr_mod[:].rearrange("p (k h e) -> p k h e", k=8, h=H)
            xnv = xr_nat[:].rearrange("p (k h c) -> p k h c", k=8, h=H)
            rep = esln[:].rearrange("p (k e) -> p k e", k=8)[:, :, 0:H]
            # broadcast esl over the 16 channels
            repb = esln[:].rearrange("p (k e one) -> p k e one", k=8, one=1)
            repb = repb[:, :, 0:H, :].broadcast_to([128, 8, H, C])
            nc.vector.tensor_tensor(xmv[:, :, :, 0:C], xnv, repb, Alu.mult)
            nc.vector.tensor_copy(xmv[:, :, :, C], rep)

        xr_mod = consts.tile([128, 8 * 68], f16, tag="xrmod")

        # ---------- main streaming loop ----------
        # st_t[ib]: S^T tiles, [j128, k*512 + s4*128 + r], r = PSUM row layout
        st_t = [consts.tile([128, 8 * 512], f16, tag=f"stt{ib}",
                            name=f"stt{ib}") for ib in range(2)]

        # ---------- aggregation ----------
        def aggregate(ib):
            out_f = outp.tile([128, HC], f32, tag="outf", name="outf")
            out_f2 = outp.tile([128, HC], f32, tag="outf2", name="outf2")
            stv = st_t[ib][:].rearrange("p (k t h) -> p k t h", k=8, h=H)
            for h in range(H):
                agg = psa.tile([128, 17], f32, tag="a", name="agg")
                for k in range(8):
                    nc.tensor.matmul(agg[:], stv[:, k, :, h],
                                     xr_mod[:, k * 68 + h * 17: k * 68 + (h + 1) * 17],
                                     start=(k == 0), stop=(k == 7))
                rz = work.tile([128, 1], f32, tag="rz", name="rz")
                nc.vector.reciprocal(rz[:], agg[:, 16:17])
                nc.vector.tensor_scalar(out_f[:, h * 16:(h + 1) * 16],
                                        agg[:, 0:16], rz[:, 0:1], None, Alu.mult)
            nc.vector.tensor_add(out_f2[:], out_f[:], biasb_t[:])
            dma(out[ib * 128:(ib + 1) * 128, :], out_f2[:])

        for sup in range(NSUP):
            ib, s4 = sup // 4, sup % 4
            if sup == 1:
                build_xr_mod()
            if sup == 4:
                aggregate(0)
            gps = psg.tile([128, N], f32, tag="g", name=f"gps{sup}")
            rps = []
            for q in range(4):
                for v in range(4):
                    p = sup * 16 + q * 4 + v
                    rp = rp_pool.tile([128, N], f16, tag="rp")
                    nc.vector.tensor_scalar(rp[:], xl2T[:], xrp[:, p:p + 1],
                                            0.0, Alu.add, Alu.max)
                    rps.append(rp)
            for half in range(2):
                s = slice(half * 512, (half + 1) * 512)
                for q in range(4):
                    for v in range(4):
                        nc.tensor.matmul(
                            gps[32 * q:32 * q + 32, s],
                            attv_t[:, 32 * v:32 * v + 32],
                            rps[q * 4 + v][:, s],
                            start=(v == 0), stop=False,
                            tile_position=(0, 32 * q),
                            skip_group_check=True,
                        )
                # additive adjacency mask over all 128 rows of this half
                nc.tensor.matmul(
                    gps[:, s], p1_t[:],
                    adjh_t[:, sup * N + half * 512: sup * N + half * 512 + 512],
                    start=False, stop=True,
                    tile_position=(0, 0),
                    skip_group_check=True,
                )
            scomp = sc_pool.tile([128, N], f16, tag="scomp")
            nc.scalar.activation(scomp[:], gps[:], Act.Exp)
            dst = st_t[ib][:].rearrange("p (k s r) -> p k s r",
                                        k=8, s=4)[:, :, s4, :]
            dmaT(dst, scomp[:])

        aggregate(1)


def _get_program():
    if "nc" not in _CACHE:
        _CACHE["nc"] = _build_program()
    return _CACHE["nc"]


def kernel(x, adj, W_l, b_l, W_r, b_r, att, bias):
    global LAST_RESULTS
    from concourse.bass_utils import run_bass_kernel_spmd

    x = np.ascontiguousarray(np.asarray(x, dtype=np.float32))
    adj = np.ascontiguousarray(np.asarray(adj, dtype=np.float32))
    W_l = np.asarray(W_l, dtype=np.float32)
    b_l = np.asarray(b_l, dtype=np.float32)
    W_r = np.asarray(W_r, dtype=np.float32)
    b_r = np.asarray(b_r, dtype=np.float32)
    att = np.asarray(att, dtype=np.float32)
    bias = np.asarray(bias, dtype=np.float32)

    # host-side constant prep
    attv = np.zeros((F, 128), np.float32)
    for v in range(4):
        for d in range(2):
            for h in range(H):
                col = 32 * v + 8 * v + 4 * d + h
                attv[d * HC + h * C:d * HC + (h + 1) * C, col] = 0.8 * att[h]
    attv = attv.astype(np.float16)
    p1 = np.zeros((32, 128), np.float16)
    for q in range(4):
        for v in range(4):
            for d in range(2):
                ld = 8 * q + 2 * v + d
                for h in range(H):
                    p1[ld, 32 * q + 8 * v + 4 * d + h] = 1.0
    attbp = np.zeros((HC, 16), np.float32)
    for h in range(H):
        attbp[h * C:(h + 1) * C, h] = att[h]
    attbp = attbp.astype(np.float16)
    biasb = np.broadcast_to(bias, (128, HC)).astype(np.float32).copy()
    blp = b_l.reshape(HC, 1).astype(np.float32).copy()
    brp = b_r.reshape(HC, 1).astype(np.float32).copy()
    wl16 = W_l.astype(np.float16).copy()
    wr16 = W_r.astype(np.float16).copy()

    in_maps = []
    for core in range(NCORES):
        b, blk = core // 4, core % 4
        i0 = blk * NI
        adjsl = adj[b, i0:i0 + NI, :].copy()
        adjsl[np.arange(NI), i0 + np.arange(NI)] = 1.0   # self loops
        # adjh2[ld, sup, j] = 30*(adj[dest,j]-1), dest = sup*32+ld
        adjh = (MASK_NEG * (adjsl.reshape(NSUP, 32, N) - 1.0)).astype(np.float16)
        adjh = np.ascontiguousarray(adjh.transpose(1, 0, 2)).reshape(32, NSUP * N)
        in_maps.append({
            "xb16": x[b].astype(np.float16),
            "xis16": x[b, i0:i0 + NI].astype(np.float16),
            "adjh2": adjh,
            "wl16": wl16, "wr16": wr16, "blp": blp, "brp": brp,
            "attv": attv, "p1m": p1, "attbp": attbp, "biasb": biasb,
        })

    nc = _get_program()
    res = run_bass_kernel_spmd(nc, in_maps, core_ids=list(range(NCORES)))
    LAST_RESULTS = res
    outp = np.zeros((B, N, HC), np.float32)
    for core in range(NCORES):
        b, blk = core // 4, core % 4
        outp[b, blk * NI:(blk + 1) * NI, :] = res.results[core]["out"]
    return outp


# revision 12
# speedup vs baseline: 1.0443x; 1.0443x over previous
# DenseGATv2Conv Trainium2 kernel (v2).
#
# Math (per batch b):
#   xl = x @ W_l + b_l ; xr = x @ W_r + b_r            [N, H*C]
#   alpha[i,j,h] = sum_c att[h,c] * leaky_relu(xl[j,hc] + xr[i,hc], 0.2)
#   S = softmax_j(alpha masked by adj(+self loops))
#   out[i,hc] = sum_j S[i,j,h] * xr[j,hc] + bias
#
# Identities used on device:
#   leaky_relu(z) = 0.2*z + 0.8*relu(z)
#   alpha[i,j,h] = 0.2*sl[j,h] + 0.2*sr[i,h] + 0.8*sum_c att[h,c]*relu(xl[j,hc]+xr[i,hc])
# exp(0.2*sr[i,h]) cancels in the softmax; exp(0.2*sl[j,h]) (= esl) is folded
# multiplicatively into the aggregation operand.  The adjacency mask is applied
# ADDITIVELY pre-exp as 30*(adj-1) accumulated into the score PSUM by a small
# matmul, so masked entries underflow to 0 in the fp16 exp output.
#
# Per core: 256 dest rows = 2 ib x 4 supers x 32 rows.  Per super the 16
# dest-row pairs all accumulate into ONE [128, 1024] PSUM tile using 4
# stationary "variants" (att columns at local offset 8v) x 4 tile positions,
# so PSUM row r = 32q + 8v + 4d + h and dest-in-core = sup*32 + 8q + 2v + d
# comes out in natural order.  One exp per super writes fp16 scores which a
# DMA crossbar transpose scatters straight into the S^T aggregation layout.
#
# Sharding: 8 cores = (batch b in 0..1) x (4 blocks of 256 destination rows).

import numpy as np

B, N, F, H, C = 2, 1024, 128, 4, 16
HC = H * C
NCORES = 8
NI = 256          # destination rows per core
NSUP = 8          # supers of 16 pairs (32 dest rows) each
MASK_NEG = 30.0   # additive mask magnitude: exp(score-30) underflows fp16

_CACHE = {}
LAST_RESULTS = None


def _build_program():
    import concourse.bass as bass
    import concourse.mybir as mybir
    import concourse.tile as tile
    from concourse import bacc

    f32 = mybir.dt.float32
    f16 = mybir.dt.float16
    Alu = mybir.AluOpType
    Act = mybir.ActivationFunctionType

    nc = bacc.Bacc(
        "TRN2",
        target_bir_lowering=False,
        debug=False,
        enable_asserts=False,
        num_devices=NCORES,
    )

    # ---- DRAM I/O ----
    xb16 = nc.dram_tensor("xb16", [N, F], f16, kind="ExternalInput").ap()
    xis16 = nc.dram_tensor("xis16", [NI, F], f16, kind="ExternalInput").ap()
    adjh2 = nc.dram_tensor("adjh2", [32, NSUP * N], f16, kind="ExternalInput").ap()
    wl16 = nc.dram_tensor("wl16", [F, HC], f16, kind="ExternalInput").ap()
    wr16 = nc.dram_tensor("wr16", [F, HC], f16, kind="ExternalInput").ap()
    blp = nc.dram_tensor("blp", [HC, 1], f32, kind="ExternalInput").ap()
    brp = nc.dram_tensor("brp", [HC, 1], f32, kind="ExternalInput").ap()
    attv = nc.dram_tensor("attv", [F, 128], f16, kind="ExternalInput").ap()
    p1m = nc.dram_tensor("p1m", [32, 128], f16, kind="ExternalInput").ap()
    attbp = nc.dram_tensor("attbp", [HC, 16], f16, kind="ExternalInput").ap()
    brpb = nc.dram_tensor("brpb", [HC, 1], f32, kind="ExternalInput").ap()
    out = nc.dram_tensor("out", [NI, HC], f32, kind="ExternalOutput").ap()

    with tile.TileContext(nc) as tc:
        _body(tc, nc, mybir, f32, f16, Alu, Act,
              xb16, xis16, adjh2, wl16, wr16, blp, brp, attv, p1m, attbp,
              brpb, out)

    nc.compile()
    return nc


def _body(tc, nc, mybir, f32, f16, Alu, Act,
          xb16, xis16, adjh2, wl16, wr16, blp, brp, attv, p1m, attbp,
          brpb, out):
    from contextlib import ExitStack
    ctx = ExitStack()
    with ctx:
        consts = ctx.enter_context(tc.tile_pool(name="consts", bufs=1))
        work = ctx.enter_context(tc.tile_pool(name="work", bufs=1))
        rp_pool = ctx.enter_context(tc.tile_pool(name="rp", bufs=32))
        sc_pool = ctx.enter_context(tc.tile_pool(name="sc", bufs=2))
        outp = ctx.enter_context(tc.tile_pool(name="outp", bufs=2))
        psg = ctx.enter_context(tc.tile_pool(name="psg", bufs=2, space="PSUM"))
        psa = ctx.enter_context(tc.tile_pool(name="psa", bufs=2, space="PSUM"))

        dma = nc.sync.dma_start
        dmaT = nc.sync.dma_start_transpose

        # All loads on one queue, ordered so the first-matmul chain (xT, wl,
        # blp, then xisT for the dest-slice projection) clears HWDGE first.
        xT = consts.tile([F, N], f16, tag="xT")       # [f, node]
        xisT = consts.tile([F, NI], f16, tag="xisT")  # [f, dest-slice node]
        wl_t = consts.tile([F, HC], f16, tag="wl")
        wr_t = consts.tile([F, HC], f16, tag="wr")
        blp_t = consts.tile([HC, 1], f32, tag="blp")
        brpb_t = consts.tile([HC, 1], f32, tag="brpb")  # b_r + bias (xr_mod)
        brp_t = consts.tile([HC, 1], f32, tag="brp")
        attv_t = consts.tile([F, 128], f16, tag="attv")
        p1_t = consts.tile([32, 128], f16, tag="p1")
        attbp_t = consts.tile([HC, 16], f16, tag="attbp")
        adjh_t = consts.tile([32, NSUP * N], f16, tag="adjh")
        dmaT(xT[:].rearrange("p (k a) -> p k a", a=128), xb16)
        dma(wl_t[:], wl16)
        dma(blp_t[:], blp)
        dmaT(xisT[:].rearrange("p (k a) -> p k a", a=128), xis16)
        dma(brp_t[:], brp)
        dma(wr_t[:], wr16)
        dma(attv_t[:], attv)
        dma(adjh_t[:], adjh2)
        dma(p1_t[:], p1m)
        dma(attbp_t[:], attbp)
        dma(brpb_t[:], brpb)

        # ---------- projections ----------
        # xl2T: (x@W_l+b_l)^T stacked twice on partitions (for pair bias adds)
        xl2T = consts.tile([128, N], f16, tag="xl2T")
        xrT16 = consts.tile([HC, N], f16, tag="xrT16")   # (x@W_r+b_r)^T
        xrsT = consts.tile([HC, NI], f32, tag="xrsT")    # dest-row slice, f32
        pj = psg.tile([HC, N], f32, tag="g", name="pj")
        for half in range(2):
            s = slice(half * 512, (half + 1) * 512)
            nc.tensor.matmul(pj[:, s], wl_t[:], xT[:, s], start=True, stop=True)
        pj3 = psa.tile([HC, NI], f32, tag="a", name="pj3")
        nc.tensor.matmul(pj3[:], wr_t[:], xisT[:], start=True, stop=True)
        nc.scalar.activation(xl2T[0:HC, :], pj[:], Act.Identity,
                             bias=blp_t[:, 0:1], scale=1.0)
        nc.scalar.activation(xl2T[HC:128, :], pj[:], Act.Identity,
                             bias=blp_t[:, 0:1], scale=1.0)
        nc.scalar.activation(xrsT[:], pj3[:], Act.Identity,
                             bias=brp_t[:, 0:1], scale=1.0)
        pj2 = psg.tile([HC, N], f32, tag="g", name="pj2")
        for half in range(2):
            s = slice(half * 512, (half + 1) * 512)
            nc.tensor.matmul(pj2[:, s], wr_t[:], xT[:, s], start=True, stop=True)
        nc.scalar.activation(xrT16[:], pj2[:], Act.Identity,
                             bias=brpb_t[:, 0:1], scale=1.0)

        # ---------- xrp: per-pair bias columns [xr[2p] ; xr[2p+1]] ----------
        xrp = consts.tile([128, 128], f32, tag="xrp")
        ev = xrsT[:].rearrange("p (a two) -> p a two", two=2)
        nc.vector.tensor_copy(xrp[0:HC, :], ev[:, :, 0])
        nc.vector.tensor_copy(xrp[HC:128, :], ev[:, :, 1])

        # ---------- xr_mod build: [j128, k, h, 0:16]=xr*esl, [..,16]=esl ----
        def build_xr_mod():
            # sl[h,j] = sum_hc att_blk[hc,h]*xl[hc,j]; esl = exp(0.2*sl)
            psl = psa.tile([16, N], f32, tag="a", name="psl")
            for half in range(2):
                s = slice(half * 512, (half + 1) * 512)
                nc.tensor.matmul(psl[:, s], attbp_t[:], xl2T[0:HC, s],
                                 start=True, stop=True)
            eslT = work.tile([16, N], f16, tag="eslT", name="eslT")
            nc.scalar.activation(eslT[:], psl[:], Act.Exp, scale=0.2)
            xr_nat = work.tile([128, 8 * HC], f16, tag="xrnat", name="xr_nat")
            esln = work.tile([128, 8 * 16], f16, tag="esln", name="esln")
            dmaT(xr_nat[:].rearrange("p (k c) -> p k c", k=8), xrT16[:])
            dmaT(esln[:].rearrange("p (k e) -> p k e", k=8), eslT[:])
            xmv = xr_mod[:].rearrange("p (k h e) -> p k h e", k=8, h=H)
            xnv = xr_nat[:].rearrange("p (k h c) -> p k h c", k=8, h=H)
            rep = esln[:].rearrange("p (k e) -> p k e", k=8)[:, :, 0:H]
            # broadcast esl over the 16 channels
            repb = esln[:].rearrange("p (k e one) -> p k e one", k=8, one=1)
            repb = repb[:, :, 0:H, :].broadcast_to([128, 8, H, C])
            nc.vector.tensor_tensor(xmv[:, :, :, 0:C], xnv, repb, Alu.mult)
            nc.vector.tensor_copy(xmv[:, :, :, C], rep)

        xr_mod = consts.tile([128, 8 * 68], f16, tag="xrmod")

        # ---------- main streaming loop ----------
        # st_t[ib]: S^T tiles, [j128, k*512 + s4*128 + r], r = PSUM row layout
        st_t = [consts.tile([128, 8 * 512], f16, tag=f"stt{ib}",
                            name=f"stt{ib}") for ib in range(2)]

        # ---------- aggregation ----------
        def aggregate(ib):
            out_f = outp.tile([128, HC], f32, tag="outf", name="outf")
            stv = st_t[ib][:].rearrange("p (k t h) -> p k t h", k=8, h=H)
            for h in range(H):
                agg = psa.tile([128, 17], f32, tag="a", name="agg")
                for k in range(8):
                    nc.tensor.matmul(agg[:], stv[:, k, :, h],
                                     xr_mod[:, k * 68 + h * 17: k * 68 + (h + 1) * 17],
                                     start=(k == 0), stop=(k == 7))
                rz = work.tile([128, 1], f32, tag="rz", name="rz")
                nc.vector.reciprocal(rz[:], agg[:, 16:17])
                nc.vector.tensor_scalar(out_f[:, h * 16:(h + 1) * 16],
                                        agg[:, 0:16], rz[:, 0:1], None, Alu.mult)
            dma(out[ib * 128:(ib + 1) * 128, :], out_f[:])

        for sup in range(NSUP):
            ib, s4 = sup // 4, sup % 4
            if sup == 1:
                build_xr_mod()
            if sup == 4:
                aggregate(0)
            gps = psg.tile([128, N], f32, tag="g", name=f"gps{sup}")
            rps = []
            for q in range(4):
                for v in range(4):
                    p = sup * 16 + q * 4 + v
                    rp = rp_pool.tile([128, N], f16, tag="rp")
                    nc.vector.tensor_scalar(rp[:], xl2T[:], xrp[:, p:p + 1],
                                            0.0, Alu.add, Alu.max)
                    rps.append(rp)
            for half in range(2):
                s = slice(half * 512, (half + 1) * 512)
                for q in range(4):
                    for v in range(4):
                        nc.tensor.matmul(
                            gps[32 * q:32 * q + 32, s],
                            attv_t[:, 32 * v:32 * v + 32],
                            rps[q * 4 + v][:, s],
                            start=(v == 0), stop=False,
                            tile_position=(0, 32 * q),
                            skip_group_check=True,
                        )
                # additive adjacency mask over all 128 rows of this half
                nc.tensor.matmul(
                    gps[:, s], p1_t[:],
                    adjh_t[:, sup * N + half * 512: sup * N + half * 512 + 512],
                    start=False, stop=True,
                    tile_position=(0, 0),
                    skip_group_check=True,
                )
            scomp = sc_pool.tile([128, N], f16, tag="scomp")
            dstv = st_t[ib][:].rearrange("p (k s r) -> p k s r",
                                         k=8, s=4)
            for half in range(2):
                s = slice(half * 512, (half + 1) * 512)
                nc.scalar.activation(scomp[:, s], gps[:, s], Act.Exp)
                dmaT(dstv[:, half * 4:(half + 1) * 4, s4, :], scomp[:, s])

        aggregate(1)


def _get_program():
    if "nc" not in _CACHE:
        _CACHE["nc"] = _build_program()
    return _CACHE["nc"]


def kernel(x, adj, W_l, b_l, W_r, b_r, att, bias):
    global LAST_RESULTS
    from concourse.bass_utils import run_bass_kernel_spmd

    x = np.ascontiguousarray(np.asarray(x, dtype=np.float32))
    adj = np.ascontiguousarray(np.asarray(adj, dtype=np.float32))
    W_l = np.asarray(W_l, dtype=np.float32)
    b_l = np.asarray(b_l, dtype=np.float32)
    W_r = np.asarray(W_r, dtype=np.float32)
    b_r = np.asarray(b_r, dtype=np.float32)
    att = np.asarray(att, dtype=np.float32)
    bias = np.asarray(bias, dtype=np.float32)

    # host-side constant prep
    attv = np.zeros((F, 128), np.float32)
    for v in range(4):
        for d in range(2):
            for h in range(H):
                col = 32 * v + 8 * v + 4 * d + h
                attv[d * HC + h * C:d * HC + (h + 1) * C, col] = 0.8 * att[h]
    attv = attv.astype(np.float16)
    p1 = np.zeros((32, 128), np.float16)
    for q in range(4):
        for v in range(4):
            for d in range(2):
                ld = 8 * q + 2 * v + d
                for h in range(H):
                    p1[ld, 32 * q + 8 * v + 4 * d + h] = 1.0
    attbp = np.zeros((HC, 16), np.float32)
    for h in range(H):
        attbp[h * C:(h + 1) * C, h] = att[h]
    attbp = attbp.astype(np.float16)
    blp = b_l.reshape(HC, 1).astype(np.float32).copy()
    brp = b_r.reshape(HC, 1).astype(np.float32).copy()
    brpb = (b_r + bias).reshape(HC, 1).astype(np.float32).copy()
    wl16 = W_l.astype(np.float16).copy()
    wr16 = W_r.astype(np.float16).copy()

    in_maps = []
    for core in range(NCORES):
        b, blk = core // 4, core % 4
        i0 = blk * NI
        adjsl = adj[b, i0:i0 + NI, :].copy()
        adjsl[np.arange(NI), i0 + np.arange(NI)] = 1.0   # self loops
        # adjh2[ld, sup, j] = 30*(adj[dest,j]-1), dest = sup*32+ld
        adjh = (MASK_NEG * (adjsl.reshape(NSUP, 32, N) - 1.0)).astype(np.float16)
        adjh = np.ascontiguousarray(adjh.transpose(1, 0, 2)).reshape(32, NSUP * N)
        in_maps.append({
            "xb16": x[b].astype(np.float16),
            "xis16": x[b, i0:i0 + NI].astype(np.float16),
            "adjh2": adjh,
            "wl16": wl16, "wr16": wr16, "blp": blp, "brp": brp,
            "attv": attv, "p1m": p1, "attbp": attbp, "brpb": brpb,
        })

    nc = _get_program()
    res = run_bass_kernel_spmd(nc, in_maps, core_ids=list(range(NCORES)))
    LAST_RESULTS = res
    outp = np.zeros((B, N, HC), np.float32)
    for core in range(NCORES):
        b, blk = core // 4, core % 4
        outp[b, blk * NI:(blk + 1) * NI, :] = res.results[core]["out"]
    return outp
